# revision 32
# baseline (speedup 1.0000x reference)
"""Bilateral filter (d=9, sigmaColor=sigmaSpace=1.5) on 8 Trainium2 NeuronCores.

Contract: kernel(img: np.ndarray[3,1024,1024] f32) -> np.ndarray[3,1024,1024] f32.

Active implementation: v4 (symmetric-pair sharing), ~133 us/core measured
slope, rel err 5.6e-3 (max-abs / max|ref|; per-element max 1.5e-2):

  - Host reflect-pads and shards H across 8 cores with a 4-row halo; each
    core holds fp16 strips: partition p (= output row p) stores padded rows
    p+4..p+8 in the free dim (only dy >= 0 rows are needed, see below), plus
    a one-column-shifted copy (B) so odd-dx taps stay 4-byte aligned for the
    DVE 2x fp16 mode.  A and B live in ONE tile so any (tap, centre) field
    pair is a single affine AP.
  - Symmetric-pair sharing: for the tap pair t=(dy,dx) / t'=(-dy,-dx) the L1
    colour distance obeys d1_{t'}[p,x] = d1_t[p-dy,x-dx], so the whole
    distance/weight field (sub, |.|, channel-sum, square, exp) is computed
    ONCE per pair.  The mirror tap's contributions are recovered inside the
    PE accumulation: num += shift(w*C0) and den += shift(w), where the
    partition shift is an off-diagonal matrix eye(128,k=dy) used as the
    matmul's stationary operand (same cost as the identity) and the x-shift
    is an AP offset.  Per pair the DVE does one subtract, two channel-sum
    adds and ONE fused product op writing [w*S | w*C0]; ACT does Abs,
    Square, Exp (exp bias = -r^2/(2 sigma^2) folds the spatial weight in).
  - PSUM is a single [den | num] region (8 banks); all accumulation via
    FD-512 matmuls (ISA cap), centre tap opens the banks.
  - Output rows p < dy get no mirror contribution from the shift matrix:
    a tiny "edge pass" computes exactly those deficit taps in a re-tiled
    layout (partition = (edge row r, 32-col block)), with per-tap validity
    (r < dy) and r^2 folded into a host-provided additive exp table; the
    sums are DMA-relayouted and merged into the main PSUM with 8 matmuls.
  - The r^2=16 ring (spatial weight g=e^{-16/4.5}=0.029) uses constant
    weights: accumulated entirely on the PE with pre-scaled matrices
    (g*I, g*SH4, 3g*I) + one "self-tap" (d1 forced to 0) in the edge pass.
    This costs ~5 us and keeps the max error ~4.7e-3 (vs 1.5e-2 if the
    ring were dropped).
  - Output: num * reciprocal_approx(den) in fp32, computed and DMA'd per
    512-column half to overlap the store.

Engine balance per pair: DVE ~6.1us busy (sub + csum + fused product), ACT
~5.2us (abs/square/exp), PE ~3.4us (16 matmuls), all overlapped by the tile
scheduler with 3-deep tile rotation.  gpsimd/Pool is intentionally UNUSED
for arithmetic: it shares the DVE's SBUF port and serializes against it
(measured +65 us).  PSUM accumulation-group resets are bank-granular —
regions sharing a bank must not open separate groups.
"""

import sys

for _p in ("/opt/trn_rl_repo",):
    if _p not in sys.path:
        sys.path.insert(0, _p)

import numpy as np

import concourse.bass as bass  # noqa: F401  (registers engine classes)
import concourse.tile as tile
from concourse import bacc, mybir
from concourse.bass_utils import run_bass_kernel_spmd

C, H, W = 3, 1024, 1024
R = 4  # kernel radius (d=9)
STD = 1.5
INV2S2 = 0.5 / (STD * STD)
N_CORES = 8
HSH = H // N_CORES  # 128 output rows per core
PW = W + 2 * R  # padded width 1032
ROWS = 2 * R + 1  # 9 rows per strip
RSTRIDE = PW + 4  # strip row stride (1036, 4B*4-aligned padding)

OFFSETS = [
    (dy, dx)
    for dy in range(-R, R + 1)
    for dx in range(-R, R + 1)
    if dy * dy + dx * dx <= R * R
]  # 49 taps

F32 = mybir.dt.float32
F16 = mybir.dt.float16
ALU = mybir.AluOpType
ACTF = mybir.ActivationFunctionType

# All 48 non-centre taps form 24 symmetric pairs sharing r^2 = dy^2+dx^2
# (the centre tap has w == 1 exactly and is folded into the PSUM group init).
# The ACT unary ops (abs/square/exp) of a pair are batched into single
# instructions — same r^2 means the exp bias is shared.  DVE ops keep their
# per-tap AP shapes: batching taps into DVE ops via a stride-2 AP dim was
# measured slower on HW (suspected 2x fp16 mode fallback; the cost model
# cannot see it).
# ordered so pairs touching rows nearest the centre come first, matching the
# DMA load order (shortens the startup critical path)
PAIRS = []
for _ady in range(0, R + 1):
    for _dy in ((0,) if _ady == 0 else (-_ady, _ady)):
        for _dx in range(1, R + 1):
            if _dy * _dy + _dx * _dx <= R * R:
                PAIRS.append(("dx", _dy, _dx))
    if _ady >= 1:
        PAIRS.append(("dy", _ady, 0))


def _build_program_fp16(reps: int = 1):
    """fp16 compute pipeline: DVE tensor_tensor ops run in 2x mode (16-bit,
    unit-stride, 4B-aligned).  Odd dx offsets are 2-byte aligned, so a
    one-element-shifted copy (B) of the strip serves those taps.

    Accumulation runs on the TensorEngine: identity-matmuls accumulate each
    tap's w*S products (and w itself for the denominator) into PSUM in fp32,
    so the vector engine only does sub / L1-sum / product per tap, and the
    accumulation carries no fp16 rounding."""
    nc = bacc.Bacc(
        "TRN2", target_bir_lowering=False, debug=False, num_devices=N_CORES
    )
    x = nc.dram_tensor("x", [C, HSH + 2 * R, PW], F16, kind="ExternalInput").ap()
    ident = nc.dram_tensor("ident", [128, 128], F16, kind="ExternalInput").ap()
    y = nc.dram_tensor("y", [C, HSH, W], F32, kind="ExternalOutput").ap()

    with tile.TileContext(nc) as tc:
        with (
            tc.tile_pool(name="strips", bufs=1) as strip_pool,
            tc.tile_pool(name="accs", bufs=1) as acc_pool,
            tc.tile_pool(name="tmps", bufs=2) as tmp_pool,
            tc.tile_pool(name="psum", bufs=1, space="PSUM") as psum_pool,
        ):
            # Load order matters for the startup critical path: the centre row
            # (4) feeds every tap's subtract, and the first pairs consume rows
            # closest to the centre — load row-major in consumption order,
            # interleaved across channels, with the shifted B rows in between.
            A = strip_pool.tile([128, C, ROWS, RSTRIDE], F16, name="stripA")
            # B = strip shifted one column left (for odd-dx taps, 4B
            # alignment); only dy in [-3, 3] has odd-dx taps -> 7 rows.
            # Loaded straight from DRAM, not copied from A.
            B = strip_pool.tile([128, C, 7, RSTRIDE], F16, name="stripB")
            for j in (4, 3, 5, 2, 6, 1, 7, 0, 8):
                for c in range(C):
                    nc.sync.dma_start(A[:, c, j, 0:PW], x[c, j : j + HSH, :])
                if 1 <= j <= 7:  # B row (j-1) mirrors A row j
                    for c in range(C):
                        nc.sync.dma_start(
                            B[:, c, j - 1, 0 : PW - 1], x[c, j : j + HSH, 1:PW]
                        )

            idt = acc_pool.tile([128, 128], F16, name="idt")
            nc.sync.dma_start(idt[:], ident[:])
            ones = acc_pool.tile([128, W], F16, name="ones")
            nc.vector.memset(ones[:], 1.0)

            # fp32 PSUM accumulators: 6 banks for num, 2 for den (16 KiB exactly)
            num_ps = psum_pool.tile([128, C, W], F32, name="num_ps")
            den_ps = psum_pool.tile([128, W], F32, name="den_ps")

            r2s = sorted({dy * dy + dx * dx for dy, dx in OFFSETS})
            bias_tiles = {}
            for r2 in r2s:
                b = acc_pool.tile([128, 1], F32, tag=f"bias{r2}", name=f"bias{r2}")
                nc.gpsimd.memset(b[:], -float(r2) * INV2S2)
                bias_tiles[r2] = b

            def S(dy, dx):
                if dx % 2 == 0:
                    return A[:, :, R + dy, R + dx : R + dx + W]
                return B[:, :, dy + 3, R - 1 + dx : R - 1 + dx + W]

            C0 = A[:, :, R, R : R + W]

            HB = W // 512  # 512-wide bank slices per channel row

            def acc_num(rhs3, start, stop):
                # num_ps[:, c, h*512:+512] (+)= I.T @ rhs3[:, c, h*512:+512]
                for c in range(C):
                    for h in range(HB):
                        nc.tensor.matmul(
                            num_ps[:, c, h * 512 : (h + 1) * 512],
                            idt[:],
                            rhs3[:, c, h * 512 : (h + 1) * 512],
                            start=start, stop=stop,
                        )

            def acc_den(rhs1, start, stop):
                for h in range(HB):
                    nc.tensor.matmul(
                        den_ps[:, h * 512 : (h + 1) * 512],
                        idt[:],
                        rhs1[:, h * 512 : (h + 1) * 512],
                        start=start, stop=stop,
                    )

            for _rep in range(reps):
                # The centre tap (0,0) has w == 1 exactly (d1 = 0, r^2 = 0):
                # it opens the PSUM accumulation groups (start=True resets).
                acc_num(C0, start=True, stop=False)
                acc_den(ones[:], start=True, stop=False)

                for pi, (kind, dy, dx) in enumerate(PAIRS):
                    last_pair = pi == len(PAIRS) - 1
                    r2 = dy * dy + dx * dx
                    taps = (
                        [(dy, -dx), (dy, dx)] if kind == "dx"
                        else [(-dy, 0), (dy, 0)]
                    )
                    dd2 = tmp_pool.tile(
                        [128, 2, C, W], F16, tag="dd2", name="dd2", bufs=2
                    )
                    for t, (tdy, tdx) in enumerate(taps):
                        nc.vector.tensor_sub(dd2[:, t], S(tdy, tdx), C0)
                    nc.scalar.activation(dd2[:], dd2[:], ACTF.Abs)
                    d1p = tmp_pool.tile(
                        [128, 2, W], F16, tag="d1p", name="d1p", bufs=2
                    )
                    # per-tap csum: batching both taps into [128,2,W] DVE ops
                    # measured slower on HW (240us vs 222us) — DVE tensor ops
                    # stay per-tap, only ACT unaries are pair-batched.
                    for t in range(2):
                        nc.vector.tensor_add(
                            d1p[:, t], dd2[:, t, 0, :], dd2[:, t, 1, :]
                        )
                        nc.vector.tensor_add(
                            d1p[:, t], d1p[:, t], dd2[:, t, 2, :]
                        )
                    q2 = tmp_pool.tile([128, 2, W], F32, tag="q2", name="q2", bufs=1)
                    nc.scalar.activation(q2[:], d1p[:], ACTF.Square)
                    w2 = tmp_pool.tile([128, 2, W], F16, tag="w2", name="w2", bufs=2)
                    nc.scalar.activation(
                        w2[:], q2[:], ACTF.Exp,
                        bias=bias_tiles[r2][:], scale=-INV2S2,
                    )
                    for t, (tdy, tdx) in enumerate(taps):
                        wb = w2[:, t].unsqueeze(1).broadcast_to([128, C, W])
                        p3 = tmp_pool.tile(
                            [128, C, W], F16, tag="p3", name="p3", bufs=3
                        )
                        nc.vector.tensor_mul(p3[:], wb, S(tdy, tdx))
                        stop = last_pair and t == 1
                        acc_num(p3, start=False, stop=stop)
                        acc_den(w2[:, t], start=False, stop=stop)

            recip = tmp_pool.tile([128, W], F32, tag="recip", name="recip", bufs=1)
            scratch = tmp_pool.tile([128, W], F32, tag="q2", name="rscratch", bufs=1)
            # den in [1, 49]: no reciprocal edge cases; ~2 ULP is plenty here
            nc.vector.reciprocal_approx_accurate(recip[:], den_ps[:], scratch[:])
            rb = recip[:].unsqueeze(1).broadcast_to([128, C, W])
            o32 = tmp_pool.tile([128, C, W], F32, name="o32", bufs=1)
            nc.vector.tensor_mul(o32[:], num_ps[:], rb)
            nc.sync.dma_start(y.rearrange("c p x -> p c x"), o32[:])

    nc.compile()
    return nc


def _build_program(reps: int = 1):
    """Build + compile the single-core SPMD Bass program."""
    nc = bacc.Bacc(
        "TRN2", target_bir_lowering=False, debug=False, num_devices=N_CORES
    )
    x = nc.dram_tensor("x", [C, HSH + 2 * R, PW], F32, kind="ExternalInput").ap()
    y = nc.dram_tensor("y", [C, HSH, W], F32, kind="ExternalOutput").ap()

    with tile.TileContext(nc) as tc:
        with (
            tc.tile_pool(name="strips", bufs=1) as strip_pool,
            tc.tile_pool(name="accs", bufs=1) as acc_pool,
            tc.tile_pool(name="tmps", bufs=2) as tmp_pool,
        ):
            strips = []
            for c in range(C):
                s = strip_pool.tile([128, ROWS, RSTRIDE], F32, tag=f"strip{c}", name=f"strip{c}")
                for j in range(ROWS):
                    nc.sync.dma_start(s[:, j, 0:PW], x[c, j : j + HSH, :])
                strips.append(s)

            num = [acc_pool.tile([128, W], F32, tag=f"num{c}", name=f"num{c}") for c in range(C)]
            den = acc_pool.tile([128, W], F32, tag="den")

            # Per-partition bias constants for the fused exp:
            # w = exp(-(d1^2 + r^2) * INV2S2)  -> bias = -r^2 * INV2S2
            r2s = sorted({dy * dy + dx * dx for dy, dx in OFFSETS})
            bias_tiles = {}
            for r2 in r2s:
                b = acc_pool.tile([128, 1], F32, tag=f"bias{r2}", name=f"bias{r2}")
                nc.gpsimd.memset(b[:], -float(r2) * INV2S2)
                bias_tiles[r2] = b

            def S(c, dy, dx):
                return strips[c][:, R + dy, R + dx : R + dx + W]

            for _rep in range(reps):
                for t in num:
                    nc.vector.memset(t[:], 0.0)
                nc.vector.memset(den[:], 0.0)

                for dy, dx in OFFSETS:
                    # diffs, channel-interleaved: dd[p, x, c] = S_c(dy,dx) - C_c
                    dd = tmp_pool.tile([128, W, C], F32, tag="dd", name="dd")
                    for c in range(C):
                        nc.vector.tensor_sub(dd[:, :, c], S(c, dy, dx), S(c, 0, 0))
                    # d1 = sum_c |dd|  (L1 color distance) in one reduce
                    d1 = tmp_pool.tile([128, W], F32, tag="d1", name="d1")
                    nc.vector.tensor_reduce(
                        d1[:], dd[:], mybir.AxisListType.X, ALU.add,
                        apply_absolute_value=True,
                    )
                    # w = exp(-(d1^2 + r^2) * INV2S2), spatial weight folded in
                    w = tmp_pool.tile([128, W], F32, tag="w", name="w")
                    nc.scalar.activation(d1[:], d1[:], ACTF.Square)
                    nc.scalar.activation(
                        w[:], d1[:], ACTF.Exp,
                        bias=bias_tiles[dy * dy + dx * dx][:], scale=-INV2S2,
                    )
                    for c in range(C):
                        p = tmp_pool.tile([128, W], F32, tag="p", name="p", bufs=3)
                        nc.vector.tensor_mul(p[:], w[:], S(c, dy, dx))
                        nc.vector.tensor_add(num[c][:], num[c][:], p[:])
                    nc.vector.tensor_add(den[:], den[:], w[:])

            recip = tmp_pool.tile([128, W], F32, tag="recip", name="recip", bufs=1)
            nc.vector.reciprocal(recip[:], den[:])
            for c in range(C):
                o = tmp_pool.tile([128, W], F32, tag="p", name="o", bufs=3)
                nc.vector.tensor_mul(o[:], num[c][:], recip[:])
                nc.sync.dma_start(y[c], o[:])

    nc.compile()
    return nc


# --------------------------------------------------------------------------
# v3: symmetric-pair sharing.
#
# For each tap pair t=(dy,dx) / t'=(-dy,-dx) the L1 colour distance obeys
#     d1_{t'}[p, x] = d1_t[p-dy, x-dx],
# so the distance field (sub/abs/csum/square/exp) is computed ONCE per pair.
# The mirror tap's contributions are recovered at accumulation time:
#     num[p] += w_t[p-dy, x-dx] * I_c[(p-dy)+4, (x-dx)+4] = G_t[p-dy, x-dx]
# where G_t = w_t o C0 (product with the CENTRE row), and the partition
# shift (p-dy) is folded into the PE accumulation by using an off-diagonal
# shift matrix as the stationary operand instead of the identity — the
# matmul costs exactly the same.  The x-shift is a free-dim AP offset.
#
# Output rows p < dy receive no mirror contribution from the shift matrix
# (their true contribution depends on the previous core's rows, present in
# the DMA'd halo).  A tiny "edge pass" computes exactly those deficit
# contributions in a re-tiled layout (partition = (edge row r, 32-col
# block b)) where the 4x1024-pixel edge region costs FD=32-per-op instead
# of FD=1024: per-tap validity (r < dy) and the r^2 spatial bias are both
# folded into a host-provided per-partition additive table for the exp.
# The edge sums are relayouted by DMA and merged into the main PSUM with
# 8 more matmuls.
#
# Engine split per pair: DVE sub + 2 products (2x fp16), ACT abs + square
# + exp, Pool (gpsimd) the two channel-sum adds, PE all accumulation.
# Dropping the r^2=16 tap ring (49 -> 45 taps, rel err 6.5e-3 vs 2e-2
# budget) removes the dy=4 row entirely.
# --------------------------------------------------------------------------

import os

IMPL = os.environ.get("BILATERAL_IMPL", "v4")
V3_R2MAX = int(os.environ.get("V3_R2MAX", "13"))
V3_ABS_DVE = int(os.environ.get("V3_ABS_DVE", "2"))  # pairs with abs on DVE
V3_CSUM = os.environ.get("V3_CSUM", "dve")  # pool | dve | dma
# pairs to drop entirely (both taps), e.g. "3:2,3:-2" -> 41-tap filter
V4_ABS = os.environ.get("V4_ABS", "act")  # act | dve
V4_GAUSS16 = os.environ.get("V4_GAUSS16", "1") == "1"  # r^2=16 ring with
# constant (spatial-only) weights, accumulated via pre-scaled PE matrices
V4_BUFS = int(os.environ.get("V4_BUFS", "3"))
V3_DROP = frozenset(
    tuple(int(v) for v in t.split(":"))
    for t in os.environ.get("V3_DROP", "").split(",") if t
)
HB = W // 512  # 512-wide PSUM bank slices


def _v3_pairs(r2max):
    ps = []
    for dy in range(0, R + 1):
        for dx in range(-R, R + 1):
            if dy == 0 and dx <= 0:
                continue
            if dy * dy + dx * dx <= r2max and (dy, dx) not in V3_DROP:
                ps.append((dy, dx))
    return ps


def _v3_edge_taps(r2max):
    """Mirror taps (-dy, e) with dy>0 whose pair is in the pair list."""
    ts = []
    for dy in range(1, R + 1):
        for e in range(-R, R + 1):
            if dy * dy + e * e <= r2max and (dy, -e) not in V3_DROP:
                ts.append((dy, e))
    return ts


def _build_program_v3(reps: int = 1, r2max: int = V3_R2MAX):
    pairs = _v3_pairs(r2max)
    etaps = _v3_edge_taps(r2max)
    NTE = len(etaps)
    maxdy = max(dy for dy, _ in pairs)
    ROWS_A = maxdy + 1  # strip rows j' = 0..maxdy (padded rows 4..4+maxdy)

    nc = bacc.Bacc(
        "TRN2", target_bir_lowering=False, debug=False, num_devices=N_CORES
    )
    x = nc.dram_tensor("x", [C, HSH + 2 * R, PW], F16, kind="ExternalInput").ap()
    mats = nc.dram_tensor(
        "mats", [maxdy + 1, 128, 128], F16, kind="ExternalInput"
    ).ap()
    ex = nc.dram_tensor("ex", [128, C, 5, 40], F16, kind="ExternalInput").ap()
    r2e = nc.dram_tensor("r2e", [128, NTE], F32, kind="ExternalInput").ap()
    y = nc.dram_tensor("y", [C, HSH, W], F32, kind="ExternalOutput").ap()
    escr_n = nc.dram_tensor("escr_n", [4, C, W], F16, kind="Internal").ap()
    escr_d = nc.dram_tensor("escr_d", [4, W], F16, kind="Internal").ap()

    with tile.TileContext(nc) as tc:
        with (
            tc.tile_pool(name="strips", bufs=1) as strip_pool,
            tc.tile_pool(name="accs", bufs=1) as acc_pool,
            tc.tile_pool(name="tmps", bufs=2) as tmp_pool,
            tc.tile_pool(name="psum", bufs=1, space="PSUM") as psum_pool,
        ):
            # --- constants + edge inputs first (edge pass overlaps strip DMA)
            mt = acc_pool.tile([128, maxdy + 1, 128], F16, name="mt")
            for d in range(maxdy + 1):
                nc.sync.dma_start(mt[:, d], mats[d])
            EX = acc_pool.tile([128, C, 5, 40], F16, name="EX")
            nc.sync.dma_start(EX[:], ex[:])
            R2E = acc_pool.tile([128, NTE], F32, name="R2E")
            nc.sync.dma_start(R2E[:], r2e[:])

            # --- strips: partition p holds padded rows p+4..p+4+maxdy
            # (row j' = dy); B = one-column-left shift for odd-offset taps.
            A = strip_pool.tile([128, C, ROWS_A, RSTRIDE], F16, name="stripA")
            B = strip_pool.tile([128, C, ROWS_A, RSTRIDE], F16, name="stripB")
            for j in range(ROWS_A):
                for c in range(C):
                    nc.sync.dma_start(A[:, c, j, 0:PW], x[c, 4 + j : 4 + j + HSH, :])
                for c in range(C):
                    nc.sync.dma_start(
                        B[:, c, j, 0 : PW - 1], x[c, 4 + j : 4 + j + HSH, 1:PW]
                    )

            ones = acc_pool.tile([128, W], F16, name="ones")
            nc.vector.memset(ones[:], 1.0)

            # fp32 PSUM accumulators: 6 banks num + 2 banks den (16 KiB).
            # The edge pass borrows den_ps[:, 0:128] before the den group
            # opens each rep (start=True resets it afterwards).
            num_ps = psum_pool.tile([128, C, W], F32, name="num_ps")
            den_ps = psum_pool.tile([128, W], F32, name="den_ps")

            r2s = sorted({dy * dy + dx * dx for dy, dx in pairs})
            bias_tiles = {}
            for r2 in r2s:
                b = acc_pool.tile([128, 1], F32, tag=f"bias{r2}", name=f"bias{r2}")
                nc.gpsimd.memset(b[:], -float(r2) * INV2S2)
                bias_tiles[r2] = b

            EXc = EX[:, :, 4, 4:36]  # centre pixels of the edge region

            for _rep in range(reps):
                # ---------------- edge pass: deficit mirror-tap sums for
                # output rows 0..3 in the (r, colblock) mini layout.
                dd_e = tmp_pool.tile([128, NTE, 3, 32], F16, tag="dd_e", name="dd_e")
                for t, (dy, e) in enumerate(etaps):
                    if V4_GAUSS16 and t == NTE - 1:
                        # gaussian self-tap: dd = 0 -> w = exp(-r2e*inv2s2)
                        nc.vector.tensor_sub(dd_e[:, t], EXc, EXc)
                    else:
                        nc.vector.tensor_sub(
                            dd_e[:, t], EX[:, :, 4 - dy, 4 + e : 36 + e], EXc
                        )
                dde_u = dd_e[:].bitcast(mybir.dt.uint16)
                nc.vector.tensor_scalar(dde_u, dde_u, 0x7FFF, None, ALU.bitwise_and)
                d1_e = tmp_pool.tile([128, NTE, 32], F16, tag="d1_e", name="d1_e")
                nc.vector.tensor_add(d1_e[:], dd_e[:, :, 0], dd_e[:, :, 1])
                nc.vector.tensor_add(d1_e[:], d1_e[:], dd_e[:, :, 2])
                q_e = tmp_pool.tile([128, NTE, 32], F32, tag="q_e", name="q_e")
                nc.scalar.activation(q_e[:], d1_e[:], ACTF.Square)
                # + r^2, + validity mask (huge value -> w=0) from host table
                nc.vector.tensor_add(
                    q_e[:], q_e[:],
                    R2E[:].unsqueeze(2).broadcast_to([128, NTE, 32]),
                )
                w_e = tmp_pool.tile([128, NTE, 32], F16, tag="w_e", name="w_e")
                nc.scalar.activation(w_e[:], q_e[:], ACTF.Exp, scale=-INV2S2)
                P_e = tmp_pool.tile([128, NTE, 3, 32], F16, tag="P_e", name="P_e")
                for t, (dy, e) in enumerate(etaps):
                    nc.vector.tensor_mul(
                        P_e[:, t],
                        w_e[:, t].unsqueeze(1).broadcast_to([128, 3, 32]),
                        EX[:, :, 3 - dy, 4 + e : 36 + e],
                    )
                eps_n = den_ps[:, 0:96]
                eps_d = den_ps[:, 512:544]  # different PSUM bank than eps_n
                for t in range(NTE):
                    st = t == 0
                    sp = t == NTE - 1
                    nc.tensor.matmul(
                        eps_n, mt[:, 0], P_e[:, t], start=st, stop=sp,
                        skip_group_check=True,
                    )
                    nc.tensor.matmul(
                        eps_d, mt[:, 0], w_e[:, t], start=st, stop=sp,
                        skip_group_check=True,
                    )
                en16 = tmp_pool.tile([128, 3, 32], F16, tag="en16", name="en16")
                nc.scalar.activation(en16[:], eps_n, ACTF.Copy)
                ed16 = tmp_pool.tile([128, 32], F16, tag="ed16", name="ed16")
                nc.scalar.activation(ed16[:], eps_d, ACTF.Copy)
                # relayout (r, b) partitions -> main rows 0..3 via DRAM
                # scratch (the DRAM AP carries the (r b) c k reorder)
                edge_nm = tmp_pool.tile([4, 3, W], F16, tag="edge_nm", name="edge_nm")
                edge_dm = tmp_pool.tile([4, W], F16, tag="edge_dm", name="edge_dm")
                for rr in range(4):
                    nc.sync.dma_start(
                        escr_n[rr].rearrange("c (b k) -> b c k", b=32),
                        en16[32 * rr : 32 * rr + 32],
                    )
                    nc.sync.dma_start(
                        escr_d[rr].rearrange("(b k) -> b k", b=32),
                        ed16[32 * rr : 32 * rr + 32],
                    )
                nc.sync.dma_start(edge_nm[:], escr_n)
                nc.sync.dma_start(edge_dm[:], escr_d)

                # ---------------- main pass
                # centre tap (w == 1) opens the PSUM groups
                for c in range(C):
                    for h in range(HB):
                        nc.tensor.matmul(
                            num_ps[:, c, h * 512 : (h + 1) * 512],
                            mt[:, 0],
                            A[:, c, 0, 4 + h * 512 : 4 + (h + 1) * 512],
                            start=True, stop=False,
                        )
                for h in range(HB):
                    nc.tensor.matmul(
                        den_ps[:, h * 512 : (h + 1) * 512],
                        mt[:, 0],
                        ones[:, h * 512 : (h + 1) * 512],
                        start=True, stop=False, skip_group_check=True,
                    )
                # edge merge into rows 0..3
                for c in range(C):
                    for h in range(HB):
                        nc.tensor.matmul(
                            num_ps[0:4, c, h * 512 : (h + 1) * 512],
                            mt[0:4, 0, 0:4],
                            edge_nm[:, c, h * 512 : (h + 1) * 512],
                            start=False, stop=False,
                        )
                for h in range(HB):
                    nc.tensor.matmul(
                        den_ps[0:4, h * 512 : (h + 1) * 512],
                        mt[0:4, 0, 0:4],
                        edge_dm[:, h * 512 : (h + 1) * 512],
                        start=False, stop=False, skip_group_check=True,
                    )

                for pi, (dy, dx) in enumerate(pairs):
                    adx = abs(dx)
                    start_x = -dx if dx > 0 else 0
                    WF = W + adx
                    WFa = WF + (WF & 1)
                    r2 = dy * dy + dx * dx
                    c0 = 4 + dx + start_x  # tap-row col offset on A
                    c1 = 4 + start_x  # centre-row col offset on A
                    S_f = (
                        A[:, :, dy, c0 : c0 + WFa]
                        if c0 % 2 == 0
                        else B[:, :, dy, c0 - 1 : c0 - 1 + WFa]
                    )
                    C_f = (
                        A[:, :, 0, c1 : c1 + WFa]
                        if c1 % 2 == 0
                        else B[:, :, 0, c1 - 1 : c1 - 1 + WFa]
                    )
                    dd = tmp_pool.tile([128, 3, WFa], F16, tag="dd", name="dd", bufs=3)
                    nc.vector.tensor_sub(dd[:], S_f, C_f)
                    if pi < V3_ABS_DVE:
                        dd_u = dd[:].bitcast(mybir.dt.uint16)
                        nc.vector.tensor_scalar(dd_u, dd_u, 0x7FFF, None, ALU.bitwise_and)
                    else:
                        nc.scalar.activation(dd[:], dd[:], ACTF.Abs)
                    if V3_CSUM == "dma":
                        # channel-sum via CCE accumulate in the DMA engines:
                        # dd[:,0] += dd[:,1]; dd[:,0] += dd[:,2] (RMW, ordered
                        # by the tile scheduler's WAW deps). d1 = dd[:, 0].
                        nc.gpsimd.dma_start(dd[:, 0], dd[:, 1], accum_op=ALU.add)
                        nc.gpsimd.dma_start(dd[:, 0], dd[:, 2], accum_op=ALU.add)
                        d1v = dd[:, 0]
                    else:
                        d1t = tmp_pool.tile([128, WFa], F16, tag="d1t", name="d1t", bufs=3)
                        d1 = tmp_pool.tile([128, WFa], F16, tag="d1", name="d1", bufs=3)
                        if V3_CSUM == "pool":
                            nc.gpsimd.tensor_tensor(d1t[:], dd[:, 0], dd[:, 1], ALU.add)
                            nc.gpsimd.tensor_tensor(d1[:], d1t[:], dd[:, 2], ALU.add)
                        else:
                            nc.vector.tensor_add(d1t[:], dd[:, 0], dd[:, 1])
                            nc.vector.tensor_add(d1[:], d1t[:], dd[:, 2])
                        d1v = d1[:]
                    q = tmp_pool.tile([128, WFa], F32, tag="q", name="q", bufs=V4_BUFS)
                    nc.scalar.activation(q[:], d1v, ACTF.Square)
                    w = tmp_pool.tile([128, WFa], F16, tag="w", name="w", bufs=3)
                    nc.scalar.activation(
                        w[:], q[:], ACTF.Exp, bias=bias_tiles[r2][:], scale=-INV2S2
                    )
                    wb = w[:].unsqueeze(1).broadcast_to([128, 3, WFa])
                    P = tmp_pool.tile([128, 3, WFa], F16, tag="P", name="P", bufs=3)
                    nc.vector.tensor_mul(P[:], wb, S_f)
                    G = tmp_pool.tile([128, 3, WFa], F16, tag="G", name="G", bufs=3)
                    nc.vector.tensor_mul(G[:], wb, C_f)
                    i0d = -start_x  # field index of x=0 for direct windows
                    i0m = -dx - start_x  # field index of x'=-dx (mirror)
                    last = pi == len(pairs) - 1
                    for c in range(C):
                        for h in range(HB):
                            o = num_ps[:, c, h * 512 : (h + 1) * 512]
                            nc.tensor.matmul(
                                o, mt[:, 0],
                                P[:, c, i0d + h * 512 : i0d + (h + 1) * 512],
                                start=False, stop=False,
                            )
                            nc.tensor.matmul(
                                o, mt[:, dy],
                                G[:, c, i0m + h * 512 : i0m + (h + 1) * 512],
                                start=False, stop=last,
                            )
                    for h in range(HB):
                        o = den_ps[:, h * 512 : (h + 1) * 512]
                        nc.tensor.matmul(
                            o, mt[:, 0],
                            w[:, i0d + h * 512 : i0d + (h + 1) * 512],
                            start=False, stop=False, skip_group_check=True,
                        )
                        nc.tensor.matmul(
                            o, mt[:, dy],
                            w[:, i0m + h * 512 : i0m + (h + 1) * 512],
                            start=False, stop=last, skip_group_check=True,
                        )

            recip = tmp_pool.tile([128, W], F32, tag="recip", name="recip", bufs=1)
            scratch = tmp_pool.tile([128, W], F32, tag="rscr", name="rscratch", bufs=1)
            nc.vector.reciprocal_approx_accurate(recip[:], den_ps[:], scratch[:])
            rb = recip[:].unsqueeze(1).broadcast_to([128, C, W])
            o32 = tmp_pool.tile([128, C, W], F32, name="o32", bufs=1)
            nc.vector.tensor_mul(o32[:], num_ps[:], rb)
            nc.sync.dma_start(y.rearrange("c p x -> p c x"), o32[:])

    nc.compile()
    return nc


def _build_program_v4(reps: int = 1, r2max: int = V3_R2MAX):
    """v3 with minimal instruction count:

    - ONE PSUM region ps[128, 4, W] = [den | num0..2]: each pair accumulates
      with 6 matmuls (direct [w|P] fused, mirror num, mirror den) instead of
      32 matmul+ldweights.  Bank resets (start=True) stay bank-aligned.
    - A/B strips in ONE tile -> the two products (w*S, w*C0) are a single
      DVE instruction writing PG[:, 1:7]; exp writes w straight into PG[:,0].
    - Edge pass: one matmul per tap (rhs [w_e|P_e]), single PSUM group.
    """
    pairs = _v3_pairs(r2max)
    etaps = _v3_edge_taps(r2max)
    if V4_GAUSS16:
        etaps = etaps + [(4, 0)]  # gaussian self-tap (dd forced to 0)
    NTE = len(etaps)
    maxdy = max(dy for dy, _ in pairs)
    ROWS_A = (R + 1) if V4_GAUSS16 else (maxdy + 1)
    NMAT = 8 if V4_GAUSS16 else (maxdy + 1)

    nc = bacc.Bacc(
        "TRN2", target_bir_lowering=False, debug=False, num_devices=N_CORES
    )
    x = nc.dram_tensor("x", [C, HSH + 2 * R, PW], F16, kind="ExternalInput").ap()
    mats = nc.dram_tensor(
        "mats", [NMAT, 128, 128], F16, kind="ExternalInput"
    ).ap()
    ex = nc.dram_tensor("ex", [128, C, 5, 40], F16, kind="ExternalInput").ap()
    r2e = nc.dram_tensor("r2e", [128, NTE], F32, kind="ExternalInput").ap()
    y = nc.dram_tensor("y", [C, HSH, W], F32, kind="ExternalOutput").ap()
    escr_n = nc.dram_tensor("escr_n", [4, C, W], F16, kind="Internal").ap()
    escr_d = nc.dram_tensor("escr_d", [4, W], F16, kind="Internal").ap()

    with tile.TileContext(nc) as tc:
        with (
            tc.tile_pool(name="strips", bufs=1) as strip_pool,
            tc.tile_pool(name="accs", bufs=1) as acc_pool,
            tc.tile_pool(name="tmps", bufs=2) as tmp_pool,
            tc.tile_pool(name="psum", bufs=1, space="PSUM") as psum_pool,
        ):
            mt = acc_pool.tile([128, NMAT, 128], F16, name="mt")
            for d in range(NMAT):
                nc.sync.dma_start(mt[:, d], mats[d])
            EX = acc_pool.tile([128, C, 5, 40], F16, name="EX")
            nc.sync.dma_start(EX[:], ex[:])
            R2E = acc_pool.tile([128, NTE], F32, name="R2E")
            nc.sync.dma_start(R2E[:], r2e[:])

            # A (slab 0) and B (slab 1 = one column left) in ONE tile so a
            # mixed-parity (S, C0) field pair is a single affine AP.
            # B row R (gauss only) unused -> not loaded.
            AB = strip_pool.tile([128, 2, C, ROWS_A, RSTRIDE], F16, name="AB")
            for j in range(ROWS_A):
                for c in range(C):
                    nc.sync.dma_start(
                        AB[:, 0, c, j, 0:PW], x[c, 4 + j : 4 + j + HSH, :]
                    )
                if j > maxdy:
                    continue
                for c in range(C):
                    nc.sync.dma_start(
                        AB[:, 1, c, j, 0 : PW - 1], x[c, 4 + j : 4 + j + HSH, 1:PW]
                    )

            ones = acc_pool.tile([128, W], F16, name="ones")
            nc.vector.memset(ones[:], 1.0)

            # unified PSUM: ps[:, 0] = den, ps[:, 1:4] = num (16 KiB = 8 banks)
            ps = psum_pool.tile([128, 4, W], F32, name="ps")

            r2s = sorted({dy * dy + dx * dx for dy, dx in pairs})
            bias_tiles = {}
            for r2 in r2s:
                b = acc_pool.tile([128, 1], F32, tag=f"bias{r2}", name=f"bias{r2}")
                nc.gpsimd.memset(b[:], -float(r2) * INV2S2)
                bias_tiles[r2] = b

            EXc = EX[:, :, 4, 4:36]

            def field_view(slab_sel, row, col, width):
                # AB[:, slab, :, row, col : col+width] as [128, 3, width]
                return AB[:, slab_sel, :, row, col : col + width]

            for _rep in range(reps):
                # ---------------- edge pass
                dd_e = tmp_pool.tile([128, NTE, 3, 32], F16, tag="dd_e", name="dd_e")
                for t, (dy, e) in enumerate(etaps):
                    if V4_GAUSS16 and t == NTE - 1:
                        # gaussian self-tap: dd = 0 -> w = exp(-r2e*inv2s2)
                        nc.vector.tensor_sub(dd_e[:, t], EXc, EXc)
                    else:
                        nc.vector.tensor_sub(
                            dd_e[:, t], EX[:, :, 4 - dy, 4 + e : 36 + e], EXc
                        )
                dde_u = dd_e[:].bitcast(mybir.dt.uint16)
                nc.vector.tensor_scalar(dde_u, dde_u, 0x7FFF, None, ALU.bitwise_and)
                d1_e = tmp_pool.tile([128, NTE, 32], F16, tag="d1_e", name="d1_e")
                nc.vector.tensor_add(d1_e[:], dd_e[:, :, 0], dd_e[:, :, 1])
                nc.vector.tensor_add(d1_e[:], d1_e[:], dd_e[:, :, 2])
                q_e = tmp_pool.tile([128, NTE, 32], F32, tag="q_e", name="q_e")
                nc.scalar.activation(q_e[:], d1_e[:], ACTF.Square)
                nc.vector.tensor_add(
                    q_e[:], q_e[:],
                    R2E[:].unsqueeze(2).broadcast_to([128, NTE, 32]),
                )
                # PEt[:, t] = [w_e | P_e0..2] -> one matmul per tap
                PEt = tmp_pool.tile([128, NTE, 4, 32], F16, tag="PEt", name="PEt")
                nc.scalar.activation(PEt[:, :, 0], q_e[:], ACTF.Exp, scale=-INV2S2)
                for t, (dy, e) in enumerate(etaps):
                    nc.vector.tensor_mul(
                        PEt[:, t, 1:4],
                        PEt[:, t, 0].unsqueeze(1).broadcast_to([128, 3, 32]),
                        EX[:, :, 4 - dy, 4 + e : 36 + e],
                    )
                eps = ps[:, 0, 0:128]  # [den(0:32) | num(32:128)] in bank 0
                for t in range(NTE):
                    nc.tensor.matmul(
                        eps, mt[:, 0], PEt[:, t], start=(t == 0), stop=(t == NTE - 1),
                        skip_group_check=True,
                    )
                en16 = tmp_pool.tile([128, 3, 32], F16, tag="en16", name="en16")
                nc.scalar.activation(en16[:], ps[:, 0, 32:128], ACTF.Copy)
                ed16 = tmp_pool.tile([128, 32], F16, tag="ed16", name="ed16")
                nc.scalar.activation(ed16[:], ps[:, 0, 0:32], ACTF.Copy)
                edge_nm = tmp_pool.tile([4, 3, W], F16, tag="edge_nm", name="edge_nm")
                edge_dm = tmp_pool.tile([4, W], F16, tag="edge_dm", name="edge_dm")
                for rr in range(4):
                    nc.sync.dma_start(
                        escr_n[rr].rearrange("c (b k) -> b c k", b=32),
                        en16[32 * rr : 32 * rr + 32],
                    )
                    nc.sync.dma_start(
                        escr_d[rr].rearrange("(b k) -> b k", b=32),
                        ed16[32 * rr : 32 * rr + 32],
                    )
                nc.sync.dma_start(edge_nm[:], escr_n)
                nc.sync.dma_start(edge_dm[:], escr_d)

                # ---------------- main pass: centre opens all banks
                # (ISA caps matmul free size at 512 = one PSUM bank)
                for h in range(HB):
                    sl = slice(h * 512, (h + 1) * 512)
                    nc.tensor.matmul(
                        ps[:, 0, sl], mt[:, 0], ones[:, sl],
                        start=True, stop=False, skip_group_check=True,
                    )
                    for c in range(C):
                        nc.tensor.matmul(
                            ps[:, 1 + c, sl], mt[:, 0],
                            AB[:, 0, c, 0, 4 + h * 512 : 4 + (h + 1) * 512],
                            start=True, stop=False, skip_group_check=True,
                        )
                # edge merge (rows 0..3)
                for h in range(HB):
                    sl = slice(h * 512, (h + 1) * 512)
                    for c in range(C):
                        nc.tensor.matmul(
                            ps[0:4, 1 + c, sl], mt[0:4, 0, 0:4], edge_nm[:, c, sl],
                            start=False, stop=False, skip_group_check=True,
                        )
                    nc.tensor.matmul(
                        ps[0:4, 0, sl], mt[0:4, 0, 0:4], edge_dm[:, sl],
                        start=False, stop=False, skip_group_check=True,
                    )

                if V4_GAUSS16:
                    # r^2=16 ring, constant weight g: mats[5] = g*I,
                    # mats[6] = g*SH4 (mirror (-4,0)), mats[7] = 3g*I (den of
                    # the three direct taps).  Mirror den is constant -> via
                    # g*SH4 on ones; rows 0..3 come from the edge self-tap.
                    g_rhs = [
                        (5, AB[:, 0, :, 0, 8 : 8 + W]),     # (0, +4)
                        (5, AB[:, 0, :, 0, 0:W]),            # (0, -4)
                        (5, AB[:, 0, :, R, 4 : 4 + W]),      # (+4, 0)
                        (6, AB[:, 0, :, 0, 4 : 4 + W]),      # (-4, 0) via SH4
                    ]
                    for h in range(HB):
                        sl = slice(h * 512, (h + 1) * 512)
                        for mi, rhs3 in g_rhs:
                            for c in range(C):
                                nc.tensor.matmul(
                                    ps[:, 1 + c, sl], mt[:, mi], rhs3[:, c, sl],
                                    start=False, stop=False, skip_group_check=True,
                                )
                        nc.tensor.matmul(
                            ps[:, 0, sl], mt[:, 7], ones[:, sl],
                            start=False, stop=False, skip_group_check=True,
                        )
                        nc.tensor.matmul(
                            ps[:, 0, sl], mt[:, 6], ones[:, sl],
                            start=False, stop=False, skip_group_check=True,
                        )

                for pi, (dy, dx) in enumerate(pairs):
                    adx = abs(dx)
                    start_x = -dx if dx > 0 else 0
                    WF = W + adx
                    WFa = WF + (WF & 1)
                    r2 = dy * dy + dx * dx
                    c0 = 4 + dx + start_x
                    c1 = 4 + start_x
                    s_slab, s_col = (0, c0) if c0 % 2 == 0 else (1, c0 - 1)
                    c_slab, c_col = (0, c1) if c1 % 2 == 0 else (1, c1 - 1)
                    S_f = field_view(s_slab, dy, s_col, WFa)
                    C_f = field_view(c_slab, 0, c_col, WFa)
                    dd = tmp_pool.tile([128, 3, WFa], F16, tag="dd", name="dd", bufs=V4_BUFS)
                    nc.vector.tensor_sub(dd[:], S_f, C_f)
                    if V4_ABS == "dve":
                        dd_u = dd[:].bitcast(mybir.dt.uint16)
                        nc.vector.tensor_scalar(dd_u, dd_u, 0x7FFF, None, ALU.bitwise_and)
                    else:
                        nc.scalar.activation(dd[:], dd[:], ACTF.Abs)
                    if V3_CSUM == "dma":
                        nc.gpsimd.dma_start(dd[:, 0], dd[:, 1], accum_op=ALU.add)
                        nc.gpsimd.dma_start(dd[:, 0], dd[:, 2], accum_op=ALU.add)
                        d1v = dd[:, 0]
                    else:
                        d1 = tmp_pool.tile([128, WFa], F16, tag="d1", name="d1", bufs=V4_BUFS)
                        nc.vector.tensor_add(d1[:], dd[:, 0], dd[:, 1])
                        nc.vector.tensor_add(d1[:], d1[:], dd[:, 2])
                        d1v = d1[:]
                    q = tmp_pool.tile([128, WFa], F32, tag="q", name="q", bufs=V4_BUFS)
                    nc.scalar.activation(q[:], d1v, ACTF.Square)
                    # PG = [w | w*S (3) | w*C0 (3)]
                    PG = tmp_pool.tile([128, 7, WFa], F16, tag="PG", name="PG", bufs=int(os.environ.get("V4_PGBUFS", "3")))
                    nc.scalar.activation(
                        PG[:, 0], q[:], ACTF.Exp, bias=bias_tiles[r2][:], scale=-INV2S2
                    )
                    # both products in ONE DVE op: in1 = [S_f | C_f] via the
                    # combined AB tile (affine two-slab view)
                    wb = PG[:, 0].unsqueeze(1).unsqueeze(1).broadcast_to([128, 2, 3, WFa])
                    in1 = _two_field_view(AB, s_slab, dy, s_col, c_slab, 0, c_col, WFa)
                    nc.vector.tensor_mul(
                        PG[:, 1:7].rearrange("p (g c) x -> p g c x", g=2), wb, in1
                    )
                    i0d = -start_x
                    i0m = -dx - start_x
                    last = pi == len(pairs) - 1
                    for h in range(HB):
                        sl = slice(h * 512, (h + 1) * 512)
                        wd = slice(i0d + h * 512, i0d + (h + 1) * 512)
                        wm = slice(i0m + h * 512, i0m + (h + 1) * 512)
                        for j in range(4):  # direct [w|P]
                            nc.tensor.matmul(
                                ps[:, j, sl], mt[:, 0], PG[:, j, wd],
                                start=False, stop=False, skip_group_check=True,
                            )
                        for c in range(C):  # mirror num (G)
                            nc.tensor.matmul(
                                ps[:, 1 + c, sl], mt[:, dy], PG[:, 4 + c, wm],
                                start=False, stop=last, skip_group_check=True,
                            )
                        nc.tensor.matmul(  # mirror den (w)
                            ps[:, 0, sl], mt[:, dy], PG[:, 0, wm],
                            start=False, stop=last, skip_group_check=True,
                        )

            recip = tmp_pool.tile([128, W], F32, tag="recip", name="recip", bufs=1)
            scratch = tmp_pool.tile([128, W], F32, tag="rscr", name="rscratch", bufs=1)
            nc.vector.reciprocal_approx_accurate(recip[:], ps[:, 0], scratch[:])
            o32 = tmp_pool.tile([128, C, W], F32, name="o32", bufs=1)
            yp = y.rearrange("c p x -> p c x")
            for h in range(HB):
                sl = slice(h * 512, (h + 1) * 512)
                rb = recip[:, sl].unsqueeze(1).broadcast_to([128, C, 512])
                nc.vector.tensor_mul(o32[:, :, sl], ps[:, 1:4, sl], rb)
                nc.sync.dma_start(yp[:, :, sl], o32[:, :, sl])

    nc.compile()
    return nc


def _stack_fields(aps):
    """Stack same-shape/stride APs on one tensor into [128, n, ...] (raw AP)."""
    if len(aps) == 1:
        return aps[0].unsqueeze(1)
    import bass_rust
    a0, a1 = aps
    d = a1.offset - a0.offset
    dims = [list(p) for p in a0.ap]
    return bass_rust.AP(
        tensor=a0.tensor, offset=a0.offset,
        ap=[dims[0], [d, 2]] + dims[1:],
    )


def _build_program_v5(reps: int = 1, r2max: int = V3_R2MAX):
    """v3 with minimal instruction count:

    - ONE PSUM region ps[128, 4, W] = [den | num0..2]: each pair accumulates
      with 6 matmuls (direct [w|P] fused, mirror num, mirror den) instead of
      32 matmul+ldweights.  Bank resets (start=True) stay bank-aligned.
    - A/B strips in ONE tile -> the two products (w*S, w*C0) are a single
      DVE instruction writing PG[:, 1:7]; exp writes w straight into PG[:,0].
    - Edge pass: one matmul per tap (rhs [w_e|P_e]), single PSUM group.
    """
    pairs = _v3_pairs(r2max)
    etaps = _v3_edge_taps(r2max)
    if V4_GAUSS16:
        etaps = etaps + [(4, 0)]  # gaussian self-tap (dd forced to 0)
    NTE = len(etaps)
    maxdy = max(dy for dy, _ in pairs)
    ROWS_A = (R + 1) if V4_GAUSS16 else (maxdy + 1)
    NMAT = 8 if V4_GAUSS16 else (maxdy + 1)

    nc = bacc.Bacc(
        "TRN2", target_bir_lowering=False, debug=False, num_devices=N_CORES
    )
    x = nc.dram_tensor("x", [C, HSH + 2 * R, PW], F16, kind="ExternalInput").ap()
    mats = nc.dram_tensor(
        "mats", [NMAT, 128, 128], F16, kind="ExternalInput"
    ).ap()
    ex = nc.dram_tensor("ex", [128, C, 5, 40], F16, kind="ExternalInput").ap()
    r2e = nc.dram_tensor("r2e", [128, NTE], F32, kind="ExternalInput").ap()
    y = nc.dram_tensor("y", [C, HSH, W], F32, kind="ExternalOutput").ap()
    escr_n = nc.dram_tensor("escr_n", [4, C, W], F16, kind="Internal").ap()
    escr_d = nc.dram_tensor("escr_d", [4, W], F16, kind="Internal").ap()

    with tile.TileContext(nc) as tc:
        with (
            tc.tile_pool(name="strips", bufs=1) as strip_pool,
            tc.tile_pool(name="accs", bufs=1) as acc_pool,
            tc.tile_pool(name="tmps", bufs=2) as tmp_pool,
            tc.tile_pool(name="psum", bufs=1, space="PSUM") as psum_pool,
        ):
            mt = acc_pool.tile([128, NMAT, 128], F16, name="mt")
            for d in range(NMAT):
                nc.sync.dma_start(mt[:, d], mats[d])
            EX = acc_pool.tile([128, C, 5, 40], F16, name="EX")
            nc.sync.dma_start(EX[:], ex[:])
            R2E = acc_pool.tile([128, NTE], F32, name="R2E")
            nc.sync.dma_start(R2E[:], r2e[:])

            # A (slab 0) and B (slab 1 = one column left) in ONE tile so a
            # mixed-parity (S, C0) field pair is a single affine AP.
            # B row R (gauss only) unused -> not loaded.
            AB = strip_pool.tile([128, 2, C, ROWS_A, RSTRIDE], F16, name="AB")
            for j in range(ROWS_A):
                for c in range(C):
                    nc.sync.dma_start(
                        AB[:, 0, c, j, 0:PW], x[c, 4 + j : 4 + j + HSH, :]
                    )
                if j > maxdy:
                    continue
                for c in range(C):
                    nc.sync.dma_start(
                        AB[:, 1, c, j, 0 : PW - 1], x[c, 4 + j : 4 + j + HSH, 1:PW]
                    )

            ones = acc_pool.tile([128, W], F16, name="ones")
            nc.vector.memset(ones[:], 1.0)

            # unified PSUM: ps[:, 0] = den, ps[:, 1:4] = num (16 KiB = 8 banks)
            ps = psum_pool.tile([128, 4, W], F32, name="ps")

            r2s = sorted({dy * dy + dx * dx for dy, dx in pairs})
            bias_tiles = {}
            for r2 in r2s:
                b = acc_pool.tile([128, 1], F32, tag=f"bias{r2}", name=f"bias{r2}")
                nc.gpsimd.memset(b[:], -float(r2) * INV2S2)
                bias_tiles[r2] = b

            EXc = EX[:, :, 4, 4:36]

            def field_view(slab_sel, row, col, width):
                # AB[:, slab, :, row, col : col+width] as [128, 3, width]
                return AB[:, slab_sel, :, row, col : col + width]

            for _rep in range(reps):
                # ---------------- edge pass
                dd_e = tmp_pool.tile([128, NTE, 3, 32], F16, tag="dd_e", name="dd_e")
                for t, (dy, e) in enumerate(etaps):
                    if V4_GAUSS16 and t == NTE - 1:
                        # gaussian self-tap: dd = 0 -> w = exp(-r2e*inv2s2)
                        nc.vector.tensor_sub(dd_e[:, t], EXc, EXc)
                    else:
                        nc.vector.tensor_sub(
                            dd_e[:, t], EX[:, :, 4 - dy, 4 + e : 36 + e], EXc
                        )
                dde_u = dd_e[:].bitcast(mybir.dt.uint16)
                nc.vector.tensor_scalar(dde_u, dde_u, 0x7FFF, None, ALU.bitwise_and)
                d1_e = tmp_pool.tile([128, NTE, 32], F16, tag="d1_e", name="d1_e")
                nc.vector.tensor_add(d1_e[:], dd_e[:, :, 0], dd_e[:, :, 1])
                nc.vector.tensor_add(d1_e[:], d1_e[:], dd_e[:, :, 2])
                q_e = tmp_pool.tile([128, NTE, 32], F32, tag="q_e", name="q_e")
                nc.scalar.activation(q_e[:], d1_e[:], ACTF.Square)
                nc.vector.tensor_add(
                    q_e[:], q_e[:],
                    R2E[:].unsqueeze(2).broadcast_to([128, NTE, 32]),
                )
                # PEt[:, t] = [w_e | P_e0..2] -> one matmul per tap
                PEt = tmp_pool.tile([128, NTE, 4, 32], F16, tag="PEt", name="PEt")
                nc.scalar.activation(PEt[:, :, 0], q_e[:], ACTF.Exp, scale=-INV2S2)
                for t, (dy, e) in enumerate(etaps):
                    nc.vector.tensor_mul(
                        PEt[:, t, 1:4],
                        PEt[:, t, 0].unsqueeze(1).broadcast_to([128, 3, 32]),
                        EX[:, :, 4 - dy, 4 + e : 36 + e],
                    )
                eps = ps[:, 0, 0:128]  # [den(0:32) | num(32:128)] in bank 0
                for t in range(NTE):
                    nc.tensor.matmul(
                        eps, mt[:, 0], PEt[:, t], start=(t == 0), stop=(t == NTE - 1),
                        skip_group_check=True,
                    )
                en16 = tmp_pool.tile([128, 3, 32], F16, tag="en16", name="en16")
                nc.scalar.activation(en16[:], ps[:, 0, 32:128], ACTF.Copy)
                ed16 = tmp_pool.tile([128, 32], F16, tag="ed16", name="ed16")
                nc.scalar.activation(ed16[:], ps[:, 0, 0:32], ACTF.Copy)
                edge_nm = tmp_pool.tile([4, 3, W], F16, tag="edge_nm", name="edge_nm")
                edge_dm = tmp_pool.tile([4, W], F16, tag="edge_dm", name="edge_dm")
                for rr in range(4):
                    nc.sync.dma_start(
                        escr_n[rr].rearrange("c (b k) -> b c k", b=32),
                        en16[32 * rr : 32 * rr + 32],
                    )
                    nc.sync.dma_start(
                        escr_d[rr].rearrange("(b k) -> b k", b=32),
                        ed16[32 * rr : 32 * rr + 32],
                    )
                nc.sync.dma_start(edge_nm[:], escr_n)
                nc.sync.dma_start(edge_dm[:], escr_d)

                # ---------------- main pass: centre opens all banks
                # (ISA caps matmul free size at 512 = one PSUM bank)
                for h in range(HB):
                    sl = slice(h * 512, (h + 1) * 512)
                    nc.tensor.matmul(
                        ps[:, 0, sl], mt[:, 0], ones[:, sl],
                        start=True, stop=False, skip_group_check=True,
                    )
                    for c in range(C):
                        nc.tensor.matmul(
                            ps[:, 1 + c, sl], mt[:, 0],
                            AB[:, 0, c, 0, 4 + h * 512 : 4 + (h + 1) * 512],
                            start=True, stop=False, skip_group_check=True,
                        )
                # edge merge (rows 0..3)
                for h in range(HB):
                    sl = slice(h * 512, (h + 1) * 512)
                    for c in range(C):
                        nc.tensor.matmul(
                            ps[0:4, 1 + c, sl], mt[0:4, 0, 0:4], edge_nm[:, c, sl],
                            start=False, stop=False, skip_group_check=True,
                        )
                    nc.tensor.matmul(
                        ps[0:4, 0, sl], mt[0:4, 0, 0:4], edge_dm[:, sl],
                        start=False, stop=False, skip_group_check=True,
                    )

                # symmetric couples (dy, +-dx) share r^2 -> their whole
                # d1/abs/csum/square/exp pipeline runs as single instructions
                groups = []
                done = set()
                for (dy, dx) in pairs:
                    if (dy, dx) in done:
                        continue
                    if dy > 0 and dx > 0 and (dy, -dx) in pairs:
                        groups.append([(dy, dx), (dy, -dx)])
                        done |= {(dy, dx), (dy, -dx)}
                    else:
                        groups.append([(dy, dx)])
                        done.add((dy, dx))

                for gi, grp in enumerate(groups):
                    n = len(grp)
                    dy0, dxa = grp[0][0], abs(grp[0][1])
                    WF = W + dxa
                    WFa = WF + (WF & 1)
                    r2 = dy0 * dy0 + dxa * dxa
                    geo = []
                    for (dy, dx) in grp:
                        start_x = -dx if dx > 0 else 0
                        c0 = 4 + dx + start_x
                        c1 = 4 + start_x
                        s_slab, s_col = (0, c0) if c0 % 2 == 0 else (1, c0 - 1)
                        c_slab, c_col = (0, c1) if c1 % 2 == 0 else (1, c1 - 1)
                        geo.append((dy, dx, start_x, s_slab, s_col, c_slab, c_col))
                    S_fs = [field_view(g[3], g[0], g[4], WFa) for g in geo]
                    C_fs = [field_view(g[5], 0, g[6], WFa) for g in geo]
                    WT = 1028  # uniform tag shape (WFa <= 1028)
                    ddf = tmp_pool.tile([128, 2, 3, WT], F16, tag="dd", name="dd", bufs=3)
                    dd = ddf[:, 0:n, :, 0:WFa]
                    nc.vector.tensor_sub(dd, _stack_fields(S_fs), _stack_fields(C_fs))
                    nc.scalar.activation(dd, dd, ACTF.Abs)
                    # csum + square in place: d1 = dd[:, :, 0]; q (fp16) = d1^2
                    d1 = ddf[:, 0:n, 0, 0:WFa]
                    nc.vector.tensor_add(d1, d1, ddf[:, 0:n, 1, 0:WFa])
                    nc.vector.tensor_add(d1, d1, ddf[:, 0:n, 2, 0:WFa])
                    nc.scalar.activation(d1, d1, ACTF.Square)
                    w2 = tmp_pool.tile([128, 2, WT], F16, tag="w2", name="w2", bufs=3)
                    nc.scalar.activation(
                        w2[:, 0:n, 0:WFa], d1, ACTF.Exp,
                        bias=bias_tiles[r2][:], scale=-INV2S2,
                    )
                    for i, (dy, dx, start_x, s_slab, s_col, c_slab, c_col) in enumerate(geo):
                        PP = tmp_pool.tile([128, 6, WT], F16, tag="PP", name="PP", bufs=3)
                        wb = w2[:, i, 0:WFa].unsqueeze(1).unsqueeze(1).broadcast_to([128, 2, 3, WFa])
                        in1 = _two_field_view(AB, s_slab, dy, s_col, c_slab, 0, c_col, WFa)
                        nc.vector.tensor_mul(
                            PP[:, 0:6, 0:WFa].rearrange("p (g c) x -> p g c x", g=2), wb, in1
                        )
                        i0d = -start_x
                        i0m = -dx - start_x
                        last = gi == len(groups) - 1 and i == n - 1
                        for h in range(HB):
                            sl = slice(h * 512, (h + 1) * 512)
                            wd = slice(i0d + h * 512, i0d + (h + 1) * 512)
                            wm = slice(i0m + h * 512, i0m + (h + 1) * 512)
                            nc.tensor.matmul(  # direct den (w)
                                ps[:, 0, sl], mt[:, 0], w2[:, i, wd],
                                start=False, stop=False, skip_group_check=True,
                            )
                            for c in range(C):  # direct num (P)
                                nc.tensor.matmul(
                                    ps[:, 1 + c, sl], mt[:, 0], PP[:, c, wd],
                                    start=False, stop=False, skip_group_check=True,
                                )
                            for c in range(C):  # mirror num (G)
                                nc.tensor.matmul(
                                    ps[:, 1 + c, sl], mt[:, dy], PP[:, 3 + c, wm],
                                    start=False, stop=last, skip_group_check=True,
                                )
                            nc.tensor.matmul(  # mirror den (w)
                                ps[:, 0, sl], mt[:, dy], w2[:, i, wm],
                                start=False, stop=last, skip_group_check=True,
                            )

            recip = tmp_pool.tile([128, W], F32, tag="recip", name="recip", bufs=1)
            scratch = tmp_pool.tile([128, W], F32, tag="rscr", name="rscratch", bufs=1)
            nc.vector.reciprocal_approx_accurate(recip[:], ps[:, 0], scratch[:])
            o32 = tmp_pool.tile([128, C, W], F32, name="o32", bufs=1)
            yp = y.rearrange("c p x -> p c x")
            for h in range(HB):
                sl = slice(h * 512, (h + 1) * 512)
                rb = recip[:, sl].unsqueeze(1).broadcast_to([128, C, 512])
                nc.vector.tensor_mul(o32[:, :, sl], ps[:, 1:4, sl], rb)
                nc.sync.dma_start(yp[:, :, sl], o32[:, :, sl])

    nc.compile()
    return nc


def _two_field_view(AB, s_slab, s_row, s_col, c_slab, c_row, c_col, width):
    """[128, 2, 3, width] AP over the combined AB strip tile: g=0 -> the
    tap field (slab s_slab, row s_row, col s_col), g=1 -> the centre field.
    Built as a raw AP with an explicit (possibly negative) g-stride."""
    import bass_rust
    base = AB[:, s_slab, :, s_row, s_col : s_col + width]  # [128, 3, width]
    other = AB[:, c_slab, :, c_row, c_col : c_col + width]
    d = other.offset - base.offset
    dims = [list(p) for p in base.ap]  # [[pstride,128],[cstride,3],[1,width]]
    return bass_rust.AP(
        tensor=base.tensor, offset=base.offset,
        ap=[dims[0], [d, 2], dims[1], dims[2]],
    )


def _shards_v3(img: np.ndarray, r2max: int = V3_R2MAX, impl: str = "v4") -> list[dict]:
    pairs = _v3_pairs(r2max)
    etaps = _v3_edge_taps(r2max)
    maxdy = max(dy for dy, _ in pairs)
    gauss = V4_GAUSS16 and impl in ("v4", "v5")
    if gauss:
        etaps = etaps + [(4, 0)]
    padded = np.pad(img, ((0, 0), (R, R), (R, R)), mode="reflect").astype(np.float16)
    eye = lambda d: np.eye(128, k=d, dtype=np.float32)
    if gauss:
        g = np.exp(-16.0 * INV2S2)
        mats = np.stack([
            eye(0), eye(1), eye(2), eye(3), eye(4),
            g * eye(0), g * eye(4), 3.0 * g * eye(0),
        ]).astype(np.float16)
    else:
        mats = np.stack([eye(d) for d in range(maxdy + 1)]).astype(np.float16)
    exrows = 5 if True else 4
    shards = []
    for i in range(N_CORES):
        xc = np.ascontiguousarray(padded[:, i * HSH : i * HSH + HSH + 2 * R, :])
        # edge mini-strip: partition p=(r*32+b) holds padded rows r..r+4,
        # cols 32b..32b+39 (32 own + 4+4 halo)
        EX = np.zeros((128, C, 5, 40), np.float16)
        for r_ in range(4):
            for b_ in range(32):
                EX[r_ * 32 + b_] = xc[:, r_ : r_ + 5, 32 * b_ : 32 * b_ + 40]
        # per-tap r^2 + validity mask (+225 -> exp(-50) = 0)
        r2e = np.zeros((128, len(etaps)), np.float32)
        for t, (dy, e) in enumerate(etaps):
            r2col = float(dy * dy + e * e)
            for p in range(128):
                r2e[p, t] = r2col if (p // 32) < dy else r2col + 225.0
        shards.append({"x": xc, "mats": mats, "ex": EX, "r2e": r2e})
    return shards

_CACHE: dict = {}


def _get_program(reps: int = 1, impl: str | None = None):
    impl = impl or IMPL
    key = (impl, reps)
    if key not in _CACHE:
        if impl == "v5":
            build = _build_program_v5
        elif impl == "v4":
            build = _build_program_v4
        elif impl == "v3":
            build = _build_program_v3
        elif impl == "fp16":
            build = _build_program_fp16
        else:
            build = _build_program
        _CACHE[key] = build(reps)
    return _CACHE[key]


def _shards(img: np.ndarray, impl: str | None = None) -> list[dict]:
    impl = impl or IMPL
    if impl in ("v3", "v4", "v5"):
        return _shards_v3(img, impl=impl)
    padded = np.pad(img, ((0, 0), (R, R), (R, R)), mode="reflect")
    if impl == "fp16":
        padded = padded.astype(np.float16)
        ident = np.eye(128, dtype=np.float16)
        return [
            {
                "x": np.ascontiguousarray(
                    padded[:, i * HSH : i * HSH + HSH + 2 * R, :]
                ),
                "ident": ident,
            }
            for i in range(N_CORES)
        ]
    return [
        {"x": np.ascontiguousarray(padded[:, i * HSH : i * HSH + HSH + 2 * R, :])}
        for i in range(N_CORES)
    ]


def kernel(img: np.ndarray) -> np.ndarray:
    img = np.asarray(img, dtype=np.float32)
    assert img.shape == (C, H, W)
    nc = _get_program()
    res = run_bass_kernel_spmd(nc, _shards(img), list(range(N_CORES))).results
    return np.concatenate([res[i]["y"] for i in range(N_CORES)], axis=1)

